# revision 39
# baseline (speedup 1.0000x reference)
# Transformer-XL style relative-position attention on 8 Trainium2 NeuronCores.
# v3.1 reconstruction (113.7us measured): phase-sequential structure.

import numpy as np

import concourse.bass as bass
import concourse.mybir as mybir
import concourse.tile as tile
from concourse import bacc, bass_utils
from concourse.tile import add_dep_helper
from contextlib import ExitStack

F32 = mybir.dt.float32
F16 = mybir.dt.float16
AF = mybir.ActivationFunctionType
OP = mybir.AluOpType

DIM = 1024
HEADS = 16
DHEAD = 64
B = 8
N = 256
M = 256
T = M + N
SCALE = DHEAD ** -0.5
NEG = -30000.0
SW = 384
NS = 257
PAD = 127


def build_kernel():
    nc = bacc.Bacc("TRN2", target_bir_lowering=False, debug=False)

    catt_d = nc.dram_tensor("catT", [DIM, T], F16, kind="ExternalInput")
    wq_d = nc.dram_tensor("wq", [DIM, DIM], F16, kind="ExternalInput")
    wk_d = nc.dram_tensor("wk", [DIM, DIM], F16, kind="ExternalInput")
    wv_d = nc.dram_tensor("wv", [DIM, DIM], F16, kind="ExternalInput")
    wo_d = nc.dram_tensor("wo", [DIM, DIM], F16, kind="ExternalInput")
    rwst_d = nc.dram_tensor("rwst", [DIM, 258], F16, kind="ExternalInput")
    uuvv_d = nc.dram_tensor("uuvv", [128, 2], F32, kind="ExternalInput")
    out_d = nc.dram_tensor("out", [N, DIM], F16, kind="ExternalOutput")
    scr_d = nc.dram_tensor("scr", [HEADS, N, SW], F16)
    junk_d = nc.dram_tensor("warm_junk", [128, 512], F16)

    with tile.TileContext(nc) as tc, ExitStack() as ctx:
        _body(ctx, tc, catt_d, wq_d, wk_d, wv_d, wo_d, rwst_d, uuvv_d,
              out_d, scr_d, junk_d)

    nc.compile()
    return nc


def _body(ctx, tc, catt_d, wq_d, wk_d, wv_d, wo_d, rwst_d, uuvv_d, out_d,
          scr_d, junk_d):
    nc = tc.nc

    const = ctx.enter_context(tc.tile_pool(name="const", bufs=1))
    persist = ctx.enter_context(tc.tile_pool(name="persist", bufs=1))
    work = ctx.enter_context(tc.tile_pool(name="work", bufs=4))
    ps_m = ctx.enter_context(tc.tile_pool(name="ps_m", bufs=4, space="PSUM"))
    ps_a = ctx.enter_context(tc.tile_pool(name="ps_a", bufs=3, space="PSUM"))
    ps_v = ctx.enter_context(tc.tile_pool(name="ps_v", bufs=1, space="PSUM"))

    junk = const.tile([128, 512], F16, tag="junk", name="junk")
    nc.vector.memset(junk, 1.0)
    uuvv = const.tile([128, 2], F32, tag="uuvv", name="uuvv_sb")

    bsb = const.tile([128, 4, 2, SW], F16, tag="bsb", name="bsb")
    nc.vector.memset(bsb, NEG)

    catt_sb = persist.tile([128, 8, T], F16, tag="catt", name="catt_sb")
    # wq_sb is chunk-major [c, dt, 256]; the host stores wq partition-major
    # within each chunk ([4][128 p][8 dt][256]) so a chunk load moves one
    # contiguous 4KB run per partition (the naive column chunk left each
    # partition 8 scattered 512B runs = 1024 min-size descriptors, and one
    # chunk took 12.3us on an otherwise idle fabric -- measured).
    wq_sb = persist.tile([128, 4, 8, 256], F16, tag="wq", name="wq_sb")
    wk_sb = persist.tile([128, 8, DIM], F16, tag="wk", name="wk_sb")
    wv_sb = persist.tile([128, 8, DIM], F16, tag="wv", name="wv_sb")
    wo_sb = persist.tile([128, 8, DIM], F16, tag="wo", name="wo_sb")
    rwst_sb = persist.tile([128, 8, 258], F16, tag="rwst", name="rwst_sb")

    def load_rows(eng, sb, dr, ncol, d0, d1, c0=0, c1=None):
        if c1 is None:
            c1 = ncol
        src = bass.AP(dr[:, 0:1].tensor, d0 * 128 * ncol + c0,
                      [[ncol, 128], [128 * ncol, d1 - d0], [1, c1 - c0]])
        return eng.dma_start(out=sb[:, d0:d1, c0:c1], in_=src)

    # q-path loads (catt_x + wq) get the bandwidth first: wq is stored
    # chunk-major by the host so each 2-ft column chunk is contiguous, and
    # catt is stored column-permuted [x|h] so each half is contiguous.
    # Everything else is gated behind the last wq chunk via explicit deps,
    # so q-proj inputs land ~7us earlier (the SDMA engines round-robin all
    # queued transfers; ungated, wq previously finished at ~26us).
    nc.sync.dma_start(out=uuvv, in_=uuvv_d[:, :])
    def load_catt(c0, dstc):
        src = bass.AP(catt_d[:, 0:1].tensor, c0,
                      [[T, 128], [128 * T, 8], [1, 256]])
        return nc.scalar.dma_start(out=catt_sb[:, :, dstc:dstc + 256], in_=src)
    load_catt(0, M)        # x tokens (stored first in DRAM)
    wq_last = None
    for c in range(4):
        src = bass.AP(wq_d[:, 0:1].tensor, c * 128 * 2048,
                      [[2048, 128], [256, 8], [1, 256]])
        wq_last = nc.sync.dma_start(out=wq_sb[:, c, :, :], in_=src)
    load_rows(nc.sync, rwst_sb, rwst_d, 258, 0, 8)
    ch = load_catt(256, 0)  # h tokens
    add_dep_helper(ch.ins, wq_last.ins, sync=True, reason="bw priority")
    load_rows(nc.scalar, wk_sb, wk_d, DIM, 0, 8)
    wv_l = load_rows(nc.gpsimd, wv_sb, wv_d, DIM, 0, 8)
    add_dep_helper(wv_l.ins, wq_last.ins, sync=True, reason="bw priority")
    load_rows(nc.gpsimd, wo_sb, wo_d, DIM, 0, 8)

    pwarm = ps_m.tile([128, 512], F32, tag="pm", name="ps_warm")
    for wi in range(12):
        nc.tensor.matmul(pwarm, junk[:, 0:128], junk,
                         start=(wi == 0), stop=(wi == 11))
    junk2 = const.tile([128, 512], F16, tag="junk2", name="junk2")
    nc.vector.tensor_copy(junk2, pwarm)
    nc.gpsimd.dma_start(out=junk_d[:, :], in_=junk2)

    quT = persist.tile([128, 8, N], F16, tag="quT", name="quT")
    qvT = persist.tile([128, 8, N], F16, tag="qvT", name="qvT")
    for ft in range(8):
        pq = ps_m.tile([128, 512], F32, tag="pm", name=f"ps_q{ft}")
        for dt in range(8):
            nc.tensor.matmul(
                pq[:, 0:N],
                wq_sb[:, ft // 2, dt, (ft % 2) * 128:(ft % 2) * 128 + 128],
                catt_sb[:, dt, M:T], start=(dt == 0), stop=(dt == 7))
        nc.vector.tensor_scalar_add(quT[:, ft, :], pq[:, 0:N], uuvv[:, 0:1])
        nc.vector.tensor_scalar_add(qvT[:, ft, :], pq[:, 0:N], uuvv[:, 1:2])

    kT = persist.tile([128, 8, T], F16, tag="kT", name="kT")
    bands = [None] * HEADS

    def bd_pair(ft):
        pbs = {}
        for qb in range(2):
            for hp in range(2):
                hh, ro = 2 * ft + hp, hp * 64
                pb = ps_m.tile([128, 512], F32, tag="pm",
                               name=f"ps_b{hh}_{qb}")
                nc.tensor.matmul(pb[:, 0:NS],
                                 qvT[ro:ro + 64, ft, qb * 128:(qb + 1) * 128],
                                 rwst_sb[ro:ro + 64, ft, 0:NS],
                                 start=True, stop=True)
                pbs[(hp, qb)] = pb
        for hp in range(2):
            hh = 2 * ft + hp
            slot = hh % 4
            for qb in range(2):
                if hp == 0:
                    nc.scalar.copy(bsb[:, slot, qb, PAD:SW],
                                   pbs[(hp, qb)][:, 0:NS])
                else:
                    nc.vector.tensor_copy(bsb[:, slot, qb, PAD:SW],
                                          pbs[(hp, qb)][:, 0:NS])
            dst = bass.AP(scr_d[0][:, 0:1].tensor, hh * N * SW,
                          [[SW, 128], [128 * SW, 2], [1, SW]])
            w = nc.gpsimd.dma_start(out=dst, in_=bsb[:, slot, :, :])
            band = work.tile([128, 2, SW], F16, tag="band", name=f"band{hh}",
                             bufs=HEADS)
            src = bass.AP(scr_d[0][:, 0:1].tensor, hh * N * SW + PAD,
                          [[SW - 1, 128], [128 * SW, 2], [1, SW]])
            r = nc.sync.dma_start(out=band[:, :, :], in_=src)
            add_dep_helper(r.ins, w.ins, sync=True, reason="scratch RAW")
            bands[hh] = band

    for ft in range(8):
        pk = ps_m.tile([128, 512], F32, tag="pm", name=f"ps_k{ft}")
        for dt in range(8):
            nc.tensor.matmul(pk, wk_sb[:, dt, ft * 128:(ft + 1) * 128],
                             catt_sb[:, dt, :], start=(dt == 0), stop=(dt == 7))
        nc.vector.tensor_copy(kT[:, ft, :], pk)
        bd_pair(ft)

    val = persist.tile([128, 4, HEADS, DHEAD], F16, tag="val", name="val")
    for jt in range(4):
        for nh in range(2):
            pv = ps_m.tile([128, 512], F32, tag="pm", name=f"ps_v{jt}_{nh}")
            for dt in range(8):
                nc.tensor.matmul(pv, catt_sb[:, dt, jt * 128:(jt + 1) * 128],
                                 wv_sb[:, dt, nh * 512:(nh + 1) * 512],
                                 start=(dt == 0), stop=(dt == 7))
            nc.vector.tensor_copy(val[:, jt, nh * 8:(nh + 1) * 8, :], pv)

    aoT = [persist.tile([128, 8, 128], F16, tag=f"aoT{qb}", name=f"aoT{qb}")
           for qb in range(2)]
    pav = ps_v.tile([128, 2, 128], F32, tag="pav", name="pav")
    po = [[None, None], [None, None]]
    osb = persist.tile([128, 2, DIM], F16, tag="osb", name="osb")
    grp = {}

    def scores_pair(ft):
        pas = {}
        for qb in range(2):
            for hp in range(2):
                hh, ro = 2 * ft + hp, hp * 64
                pa = ps_a.tile([128, SW], F32, tag="pa", name=f"ps_a{hh}_{qb}")
                nc.tensor.matmul(pa,
                                 quT[ro:ro + 64, ft, qb * 128:(qb + 1) * 128],
                                 kT[ro:ro + 64, ft, qb * 128:qb * 128 + SW],
                                 start=True, stop=True)
                pas[(hp, qb)] = pa
        for hp in range(2):
            hh = 2 * ft + hp
            band = bands[hh]
            att = work.tile([128, 2, SW], F16, tag="att", name=f"att{hh}",
                            bufs=4)
            attp = work.tile([128, 2, SW], F16, tag="attp", name=f"attp{hh}",
                             bufs=4)
            rcp = work.tile([128, 2], F32, tag="rcp", name=f"rcp{hh}", bufs=4)
            for qb in range(2):
                nc.vector.scalar_tensor_tensor(attp[:, qb, :], pas[(hp, qb)],
                                               1.0, band[:, qb, :],
                                               OP.mult, OP.add)
                ssum = work.tile([128, 1], F32, tag="ssum",
                                 name=f"ss{hh}_{qb}", bufs=12)
                nc.scalar.activation(att[:, qb, :], attp[:, qb, :], AF.Exp,
                                     bias=0.0, scale=SCALE, accum_out=ssum)
                nc.vector.reciprocal(rcp[:, qb:qb + 1], ssum)
                if hp == 0:
                    nc.vector.tensor_scalar_mul(att[:, qb, :], att[:, qb, :],
                                                rcp[:, qb:qb + 1])
                else:
                    nc.scalar.activation(att[:, qb, :], att[:, qb, :],
                                         AF.Copy, bias=0.0,
                                         scale=rcp[:, qb:qb + 1])
            attT = work.tile([128, 6, 128], F16, tag="attT", name=f"attT{hh}",
                             bufs=6)
            nc.sync.dma_start(out=attT[:, :, :], in_=att[:, 0:2, :],
                              transpose=True)
            grp[hh] = attT

    def av_pair(ft):
        for qb in range(2):
            for hp in range(2):
                hh = 2 * ft + hp
                for w in range(3):
                    nc.tensor.matmul(pav[hp * 64:hp * 64 + 64, qb, :],
                                     val[:, qb + w, hh, :],
                                     grp[hh][:, qb * 3 + w, :],
                                     start=(w == 0), stop=(w == 2))
        for qb in range(2):
            nc.vector.tensor_copy(aoT[qb][:, ft, :], pav[:, qb, :])
        for qb in range(2):
            for nh in range(2):
                if ft == 0:
                    po[qb][nh] = ps_m.tile([128, 512], F32, tag="pm",
                                           name=f"ps_o{qb}_{nh}")
                nc.tensor.matmul(po[qb][nh], aoT[qb][:, ft, :],
                                 wo_sb[:, ft, nh * 512:(nh + 1) * 512],
                                 start=(ft == 0), stop=(ft == 7))

    # two-pair-deep software pipeline: av runs two score groups behind, so
    # the exp->normalize->transpose latency of a pair hides under the next
    # pair's scores AND the previous pair's AV (one-deep left a 5.4us PE gap
    # at the v->attention transition -- measured).
    scores_pair(0)
    scores_pair(1)
    for ft in range(2, 8):
        scores_pair(ft)
        av_pair(ft - 2)
    av_pair(6)
    av_pair(7)

    for qb in range(2):
        nc.scalar.copy(osb[:, qb, 0:512], po[qb][0])
        nc.vector.tensor_copy(osb[:, qb, 512:1024], po[qb][1])
        dst = bass.AP(out_d[:, 0:1].tensor, qb * 128 * DIM,
                      [[DIM, 128], [1, DIM]])
        nc.sync.dma_start(out=dst, in_=osb[:, qb, :])


def host_prep(inputs):
    x = np.asarray(inputs["x"], dtype=np.float32)
    h = np.asarray(inputs["h"], dtype=np.float32)
    wqkv = np.asarray(inputs["Wqkv"], dtype=np.float32)
    wkr = np.asarray(inputs["Wkr"], dtype=np.float32)
    r = np.asarray(inputs["R"], dtype=np.float32)
    u = np.asarray(inputs["u"], dtype=np.float32)
    v = np.asarray(inputs["v"], dtype=np.float32)
    wout = np.asarray(inputs["Wout"], dtype=np.float32)

    # wq stored [chunk c][partition p][dt][256 col] so each chunk load moves
    # one contiguous 4KB run per SBUF partition.
    wq_f = wqkv[:, 0:DIM].astype(np.float16)       # [d, col]
    wq = np.ascontiguousarray(
        wq_f.reshape(8, 128, 4, 256).transpose(2, 1, 0, 3)
    ).reshape(DIM, DIM)
    wk = np.ascontiguousarray(wqkv[:, DIM:2 * DIM].astype(np.float16))
    wv = np.ascontiguousarray(wqkv[:, 2 * DIM:3 * DIM].astype(np.float16))
    wo = np.ascontiguousarray(wout.astype(np.float16))

    rows = (np.arange(NS) + 768) % 1024
    rws = r[rows] @ wkr
    rwst = np.zeros((DIM, 258), dtype=np.float16)
    rwst[:, 0:NS] = rws.T.astype(np.float16)

    uuvv = np.stack([np.tile(u, 2), np.tile(v, 2)], axis=1)
    uuvv = np.ascontiguousarray(uuvv.astype(np.float32))

    catts = []
    for b in range(B):
        cat = np.concatenate([h[b], x[b]], axis=0)
        catT = cat.T.astype(np.float16)
        # stored column-permuted [x|h] so each half loads contiguously
        catts.append(np.ascontiguousarray(
            np.concatenate([catT[:, M:T], catT[:, 0:M]], axis=1)))

    shared = {"wq": wq, "wk": wk, "wv": wv, "wo": wo, "rwst": rwst,
              "uuvv": uuvv}
    return catts, shared


_NC_CACHE = {}


def _get_nc():
    if "nc" not in _NC_CACHE:
        _NC_CACHE["nc"] = build_kernel()
    return _NC_CACHE["nc"]


def _run(inputs, trace=False):
    catts, shared = host_prep(inputs)
    nc = _get_nc()
    in_maps = [dict(shared, catT=catts[b]) for b in range(B)]
    res = bass_utils.run_bass_kernel_spmd(
        nc, in_maps, core_ids=list(range(B)), trace=trace)
    out = np.stack([res.results[b]["out"].astype(np.float32)
                    for b in range(B)])
    return out, res


def kernel(**inputs):
    out, _ = _run(inputs, trace=False)
    return out


# revision 40
# speedup vs baseline: 1.0348x; 1.0348x over previous
# Transformer-XL style relative-position attention on 8 Trainium2 NeuronCores.
# v3.1 reconstruction (113.7us measured): phase-sequential structure.

import numpy as np

import concourse.bass as bass
import concourse.mybir as mybir
import concourse.tile as tile
from concourse import bacc, bass_utils
from concourse.tile import add_dep_helper
from contextlib import ExitStack

F32 = mybir.dt.float32
F16 = mybir.dt.float16
AF = mybir.ActivationFunctionType
OP = mybir.AluOpType

DIM = 1024
HEADS = 16
DHEAD = 64
B = 8
N = 256
M = 256
T = M + N
SCALE = DHEAD ** -0.5
NEG = -30000.0
SW = 384
NS = 257
PAD = 127


def build_kernel():
    nc = bacc.Bacc("TRN2", target_bir_lowering=False, debug=False)

    catt_d = nc.dram_tensor("catT", [DIM, T], F16, kind="ExternalInput")
    wq_d = nc.dram_tensor("wq", [DIM, DIM], F16, kind="ExternalInput")
    wk_d = nc.dram_tensor("wk", [DIM, DIM], F16, kind="ExternalInput")
    wv_d = nc.dram_tensor("wv", [DIM, DIM], F16, kind="ExternalInput")
    wo_d = nc.dram_tensor("wo", [DIM, DIM], F16, kind="ExternalInput")
    rwst_d = nc.dram_tensor("rwst", [DIM, 258], F16, kind="ExternalInput")
    uuvv_d = nc.dram_tensor("uuvv", [128, 2], F32, kind="ExternalInput")
    out_d = nc.dram_tensor("out", [N, DIM], F16, kind="ExternalOutput")
    scr_d = nc.dram_tensor("scr", [HEADS, N, SW], F16)
    junk_d = nc.dram_tensor("warm_junk", [128, 512], F16)

    with tile.TileContext(nc) as tc, ExitStack() as ctx:
        _body(ctx, tc, catt_d, wq_d, wk_d, wv_d, wo_d, rwst_d, uuvv_d,
              out_d, scr_d, junk_d)

    nc.compile()
    return nc


def _body(ctx, tc, catt_d, wq_d, wk_d, wv_d, wo_d, rwst_d, uuvv_d, out_d,
          scr_d, junk_d):
    nc = tc.nc

    const = ctx.enter_context(tc.tile_pool(name="const", bufs=1))
    persist = ctx.enter_context(tc.tile_pool(name="persist", bufs=1))
    work = ctx.enter_context(tc.tile_pool(name="work", bufs=4))
    ps_m = ctx.enter_context(tc.tile_pool(name="ps_m", bufs=4, space="PSUM"))
    ps_a = ctx.enter_context(tc.tile_pool(name="ps_a", bufs=3, space="PSUM"))
    ps_v = ctx.enter_context(tc.tile_pool(name="ps_v", bufs=1, space="PSUM"))

    junk = const.tile([128, 512], F16, tag="junk", name="junk")
    nc.vector.memset(junk, 1.0)
    uuvv = const.tile([128, 2], F32, tag="uuvv", name="uuvv_sb")

    bsb = const.tile([128, 4, 2, SW], F16, tag="bsb", name="bsb")
    nc.vector.memset(bsb, NEG)

    catt_sb = persist.tile([128, 8, T], F16, tag="catt", name="catt_sb")
    # wq_sb is chunk-major [c, dt, 256]; the host stores wq partition-major
    # within each chunk ([4][128 p][8 dt][256]) so a chunk load moves one
    # contiguous 4KB run per partition (the naive column chunk left each
    # partition 8 scattered 512B runs = 1024 min-size descriptors, and one
    # chunk took 12.3us on an otherwise idle fabric -- measured).
    wq_sb = persist.tile([128, 4, 8, 256], F16, tag="wq", name="wq_sb")
    wk_sb = persist.tile([128, 8, DIM], F16, tag="wk", name="wk_sb")
    wv_sb = persist.tile([128, 8, DIM], F16, tag="wv", name="wv_sb")
    wo_sb = persist.tile([128, 8, DIM], F16, tag="wo", name="wo_sb")
    rwst_sb = persist.tile([128, 8, 258], F16, tag="rwst", name="rwst_sb")

    def load_rows(eng, sb, dr, ncol, d0, d1, c0=0, c1=None):
        if c1 is None:
            c1 = ncol
        src = bass.AP(dr[:, 0:1].tensor, d0 * 128 * ncol + c0,
                      [[ncol, 128], [128 * ncol, d1 - d0], [1, c1 - c0]])
        return eng.dma_start(out=sb[:, d0:d1, c0:c1], in_=src)

    # q-path loads (catt_x + wq) get the bandwidth first: wq is stored
    # chunk-major by the host so each 2-ft column chunk is contiguous, and
    # catt is stored column-permuted [x|h] so each half is contiguous.
    # Everything else is gated behind the last wq chunk via explicit deps,
    # so q-proj inputs land ~7us earlier (the SDMA engines round-robin all
    # queued transfers; ungated, wq previously finished at ~26us).
    nc.sync.dma_start(out=uuvv, in_=uuvv_d[:, :])
    def load_catt(c0, dstc):
        src = bass.AP(catt_d[:, 0:1].tensor, c0,
                      [[T, 128], [128 * T, 8], [1, 256]])
        return nc.scalar.dma_start(out=catt_sb[:, :, dstc:dstc + 256], in_=src)
    # One DMA stream moves ~130GB/s; the ~430GB/s fabric rate needs many
    # concurrent streams (measured: a lone gated 2MB wq took ~16us).  So wq
    # goes as 8 quarter-chunk streams alternating across BOTH HWDGE rings,
    # wk as dt-halves behind them, catt_h next (FIFO placement is the
    # priority mechanism); only wv/wo (SWDGE) need an explicit gate.
    load_catt(0, M)        # x tokens (stored first in DRAM)
    wq_last = None
    for c in range(4):
        for hf in range(2):
            src = bass.AP(wq_d[:, 0:1].tensor, c * 128 * 2048 + hf * 1024,
                          [[2048, 128], [256, 4], [1, 256]])
            eng = nc.sync if hf == 0 else nc.scalar
            wq_last = eng.dma_start(
                out=wq_sb[:, c, hf * 4:(hf + 1) * 4, :], in_=src)
    load_rows(nc.sync, wk_sb, wk_d, DIM, 0, 4)
    load_rows(nc.scalar, wk_sb, wk_d, DIM, 4, 8)
    load_catt(256, 0)      # h tokens
    load_rows(nc.sync, rwst_sb, rwst_d, 258, 0, 8)
    wv_l = load_rows(nc.gpsimd, wv_sb, wv_d, DIM, 0, 8)
    add_dep_helper(wv_l.ins, wq_last.ins, sync=True, reason="bw priority")
    load_rows(nc.gpsimd, wo_sb, wo_d, DIM, 0, 8)

    pwarm = ps_m.tile([128, 512], F32, tag="pm", name="ps_warm")
    for wi in range(12):
        nc.tensor.matmul(pwarm, junk[:, 0:128], junk,
                         start=(wi == 0), stop=(wi == 11))
    junk2 = const.tile([128, 512], F16, tag="junk2", name="junk2")
    nc.vector.tensor_copy(junk2, pwarm)
    nc.gpsimd.dma_start(out=junk_d[:, :], in_=junk2)

    quT = persist.tile([128, 8, N], F16, tag="quT", name="quT")
    qvT = persist.tile([128, 8, N], F16, tag="qvT", name="qvT")
    for ft in range(8):
        pq = ps_m.tile([128, 512], F32, tag="pm", name=f"ps_q{ft}")
        for dt in range(8):
            nc.tensor.matmul(
                pq[:, 0:N],
                wq_sb[:, ft // 2, dt, (ft % 2) * 128:(ft % 2) * 128 + 128],
                catt_sb[:, dt, M:T], start=(dt == 0), stop=(dt == 7))
        nc.vector.tensor_scalar_add(quT[:, ft, :], pq[:, 0:N], uuvv[:, 0:1])
        nc.vector.tensor_scalar_add(qvT[:, ft, :], pq[:, 0:N], uuvv[:, 1:2])

    kT = persist.tile([128, 8, T], F16, tag="kT", name="kT")
    bands = [None] * HEADS

    def bd_pair(ft):
        pbs = {}
        for qb in range(2):
            for hp in range(2):
                hh, ro = 2 * ft + hp, hp * 64
                pb = ps_m.tile([128, 512], F32, tag="pm",
                               name=f"ps_b{hh}_{qb}")
                nc.tensor.matmul(pb[:, 0:NS],
                                 qvT[ro:ro + 64, ft, qb * 128:(qb + 1) * 128],
                                 rwst_sb[ro:ro + 64, ft, 0:NS],
                                 start=True, stop=True)
                pbs[(hp, qb)] = pb
        for hp in range(2):
            hh = 2 * ft + hp
            slot = hh % 4
            for qb in range(2):
                if hp == 0:
                    nc.scalar.copy(bsb[:, slot, qb, PAD:SW],
                                   pbs[(hp, qb)][:, 0:NS])
                else:
                    nc.vector.tensor_copy(bsb[:, slot, qb, PAD:SW],
                                          pbs[(hp, qb)][:, 0:NS])
            dst = bass.AP(scr_d[0][:, 0:1].tensor, hh * N * SW,
                          [[SW, 128], [128 * SW, 2], [1, SW]])
            w = nc.gpsimd.dma_start(out=dst, in_=bsb[:, slot, :, :])
            band = work.tile([128, 2, SW], F16, tag="band", name=f"band{hh}",
                             bufs=HEADS)
            src = bass.AP(scr_d[0][:, 0:1].tensor, hh * N * SW + PAD,
                          [[SW - 1, 128], [128 * SW, 2], [1, SW]])
            r = nc.sync.dma_start(out=band[:, :, :], in_=src)
            add_dep_helper(r.ins, w.ins, sync=True, reason="scratch RAW")
            bands[hh] = band

    for ft in range(8):
        pk = ps_m.tile([128, 512], F32, tag="pm", name=f"ps_k{ft}")
        for dt in range(8):
            nc.tensor.matmul(pk, wk_sb[:, dt, ft * 128:(ft + 1) * 128],
                             catt_sb[:, dt, :], start=(dt == 0), stop=(dt == 7))
        nc.vector.tensor_copy(kT[:, ft, :], pk)
        bd_pair(ft)

    val = persist.tile([128, 4, HEADS, DHEAD], F16, tag="val", name="val")
    for jt in range(4):
        for nh in range(2):
            pv = ps_m.tile([128, 512], F32, tag="pm", name=f"ps_v{jt}_{nh}")
            for dt in range(8):
                nc.tensor.matmul(pv, catt_sb[:, dt, jt * 128:(jt + 1) * 128],
                                 wv_sb[:, dt, nh * 512:(nh + 1) * 512],
                                 start=(dt == 0), stop=(dt == 7))
            nc.vector.tensor_copy(val[:, jt, nh * 8:(nh + 1) * 8, :], pv)

    aoT = [persist.tile([128, 8, 128], F16, tag=f"aoT{qb}", name=f"aoT{qb}")
           for qb in range(2)]
    pav = ps_v.tile([128, 2, 128], F32, tag="pav", name="pav")
    po = [[None, None], [None, None]]
    osb = persist.tile([128, 2, DIM], F16, tag="osb", name="osb")
    grp = {}

    def scores_pair(ft):
        pas = {}
        for qb in range(2):
            for hp in range(2):
                hh, ro = 2 * ft + hp, hp * 64
                pa = ps_a.tile([128, SW], F32, tag="pa", name=f"ps_a{hh}_{qb}")
                nc.tensor.matmul(pa,
                                 quT[ro:ro + 64, ft, qb * 128:(qb + 1) * 128],
                                 kT[ro:ro + 64, ft, qb * 128:qb * 128 + SW],
                                 start=True, stop=True)
                pas[(hp, qb)] = pa
        for hp in range(2):
            hh = 2 * ft + hp
            band = bands[hh]
            att = work.tile([128, 2, SW], F16, tag="att", name=f"att{hh}",
                            bufs=4)
            attp = work.tile([128, 2, SW], F16, tag="attp", name=f"attp{hh}",
                             bufs=4)
            rcp = work.tile([128, 2], F32, tag="rcp", name=f"rcp{hh}", bufs=4)
            for qb in range(2):
                nc.vector.scalar_tensor_tensor(attp[:, qb, :], pas[(hp, qb)],
                                               1.0, band[:, qb, :],
                                               OP.mult, OP.add)
                ssum = work.tile([128, 1], F32, tag="ssum",
                                 name=f"ss{hh}_{qb}", bufs=12)
                nc.scalar.activation(att[:, qb, :], attp[:, qb, :], AF.Exp,
                                     bias=0.0, scale=SCALE, accum_out=ssum)
                nc.vector.reciprocal(rcp[:, qb:qb + 1], ssum)
                if hp == 0:
                    nc.vector.tensor_scalar_mul(att[:, qb, :], att[:, qb, :],
                                                rcp[:, qb:qb + 1])
                else:
                    nc.scalar.activation(att[:, qb, :], att[:, qb, :],
                                         AF.Copy, bias=0.0,
                                         scale=rcp[:, qb:qb + 1])
            attT = work.tile([128, 6, 128], F16, tag="attT", name=f"attT{hh}",
                             bufs=6)
            nc.sync.dma_start(out=attT[:, :, :], in_=att[:, 0:2, :],
                              transpose=True)
            grp[hh] = attT

    def av_pair(ft):
        for qb in range(2):
            for hp in range(2):
                hh = 2 * ft + hp
                for w in range(3):
                    nc.tensor.matmul(pav[hp * 64:hp * 64 + 64, qb, :],
                                     val[:, qb + w, hh, :],
                                     grp[hh][:, qb * 3 + w, :],
                                     start=(w == 0), stop=(w == 2))
        for qb in range(2):
            nc.vector.tensor_copy(aoT[qb][:, ft, :], pav[:, qb, :])
        for qb in range(2):
            for nh in range(2):
                if ft == 0:
                    po[qb][nh] = ps_m.tile([128, 512], F32, tag="pm",
                                           name=f"ps_o{qb}_{nh}")
                nc.tensor.matmul(po[qb][nh], aoT[qb][:, ft, :],
                                 wo_sb[:, ft, nh * 512:(nh + 1) * 512],
                                 start=(ft == 0), stop=(ft == 7))

    # two-pair-deep software pipeline: av runs two score groups behind, so
    # the exp->normalize->transpose latency of a pair hides under the next
    # pair's scores AND the previous pair's AV (one-deep left a 5.4us PE gap
    # at the v->attention transition -- measured).
    scores_pair(0)
    scores_pair(1)
    for ft in range(2, 8):
        scores_pair(ft)
        av_pair(ft - 2)
    av_pair(6)
    av_pair(7)

    for qb in range(2):
        nc.scalar.copy(osb[:, qb, 0:512], po[qb][0])
        nc.vector.tensor_copy(osb[:, qb, 512:1024], po[qb][1])
        dst = bass.AP(out_d[:, 0:1].tensor, qb * 128 * DIM,
                      [[DIM, 128], [1, DIM]])
        nc.sync.dma_start(out=dst, in_=osb[:, qb, :])


def host_prep(inputs):
    x = np.asarray(inputs["x"], dtype=np.float32)
    h = np.asarray(inputs["h"], dtype=np.float32)
    wqkv = np.asarray(inputs["Wqkv"], dtype=np.float32)
    wkr = np.asarray(inputs["Wkr"], dtype=np.float32)
    r = np.asarray(inputs["R"], dtype=np.float32)
    u = np.asarray(inputs["u"], dtype=np.float32)
    v = np.asarray(inputs["v"], dtype=np.float32)
    wout = np.asarray(inputs["Wout"], dtype=np.float32)

    # wq stored [chunk c][partition p][dt][256 col] so each chunk load moves
    # one contiguous 4KB run per SBUF partition.
    wq_f = wqkv[:, 0:DIM].astype(np.float16)       # [d, col]
    wq = np.ascontiguousarray(
        wq_f.reshape(8, 128, 4, 256).transpose(2, 1, 0, 3)
    ).reshape(DIM, DIM)
    wk = np.ascontiguousarray(wqkv[:, DIM:2 * DIM].astype(np.float16))
    wv = np.ascontiguousarray(wqkv[:, 2 * DIM:3 * DIM].astype(np.float16))
    wo = np.ascontiguousarray(wout.astype(np.float16))

    rows = (np.arange(NS) + 768) % 1024
    rws = r[rows] @ wkr
    rwst = np.zeros((DIM, 258), dtype=np.float16)
    rwst[:, 0:NS] = rws.T.astype(np.float16)

    uuvv = np.stack([np.tile(u, 2), np.tile(v, 2)], axis=1)
    uuvv = np.ascontiguousarray(uuvv.astype(np.float32))

    catts = []
    for b in range(B):
        cat = np.concatenate([h[b], x[b]], axis=0)
        catT = cat.T.astype(np.float16)
        # stored column-permuted [x|h] so each half loads contiguously
        catts.append(np.ascontiguousarray(
            np.concatenate([catT[:, M:T], catT[:, 0:M]], axis=1)))

    shared = {"wq": wq, "wk": wk, "wv": wv, "wo": wo, "rwst": rwst,
              "uuvv": uuvv}
    return catts, shared


_NC_CACHE = {}


def _get_nc():
    if "nc" not in _NC_CACHE:
        _NC_CACHE["nc"] = build_kernel()
    return _NC_CACHE["nc"]


def _run(inputs, trace=False):
    catts, shared = host_prep(inputs)
    nc = _get_nc()
    in_maps = [dict(shared, catT=catts[b]) for b in range(B)]
    res = bass_utils.run_bass_kernel_spmd(
        nc, in_maps, core_ids=list(range(B)), trace=trace)
    out = np.stack([res.results[b]["out"].astype(np.float32)
                    for b in range(B)])
    return out, res


def kernel(**inputs):
    out, _ = _run(inputs, trace=False)
    return out
